# revision 7
# baseline (speedup 1.0000x reference)
"""GAT (2-layer, PyG-default) Trainium2 Bass kernel, 8-core SPMD.

v2 — destination-major edge layout:
  - Nodes are permuted so each core's 6272 dst nodes are sorted by
    in-degree; the node table T1 is stored in this permuted order.  A
    chunk = 128 consecutive permuted dsts (uniform degree), one per
    SBUF partition.  Edges of dst p sit at [partition p, slot k] of the
    chunk's gather tile, so the edge->dst scatter matrix is the
    IDENTITY: aggregation is one accumulating PE matmul per 128-edge
    slot, and softmax (logits, leakyrelu, exp, masking, z) is pure
    elementwise DVE/ACT work.  No per-token transposes or selection
    matrices.
  - Phase 0 (replicated): T1[pos, :] = [h1(512) | al_src f32(8)] from
    x @ [W1 | W1@Asrc], batched 512 rows per DMA; al_dst kept on-chip.
  - int16 gather indices span only 32768 rows, so each slot is bound
    to one of W=4 overlapping 32768-row windows; a host-side greedy
    (Hall prefix/suffix sizing) assigns each dst's edges to slots.
    Self-loops are ordinary edges.  Pad slots gather window base row 0
    and are zeroed via a {0,1} mask multiplied into exp(logit).
  - L1 chunk result -> relu -> fused W2_ext projection -> tb2 row
    (40 cls | al2_src | al2_dst as f32 pairs); AllGather shares tables;
    L2 repeats with 256B rows and DVE-only aggregation (40 cols).

Self-contained: only needs numpy + the concourse tree at /opt/trn_rl_repo.
"""

import hashlib
import sys

import numpy as np

for _p in ("/opt/trn_rl_repo",):
    if _p not in sys.path:
        sys.path.insert(0, _p)

import concourse.bacc as bacc
import concourse.bass as bass
import concourse.tile as tile
from concourse import mybir
from concourse.bass_utils import run_bass_kernel_spmd

F32 = mybir.dt.float32
BF16 = mybir.dt.bfloat16
I16 = mybir.dt.int16
AF = mybir.ActivationFunctionType
OP = mybir.AluOpType
AX = mybir.AxisListType

N_CORES = 8
SPAN = 32768
W = 4
GMAX = 8
_QCTR = [0]  # global SWDGE queue round-robin


# ----------------------------------------------------------------------------
# Host-side edge planning
# ----------------------------------------------------------------------------

def _edge_plan(edge_index, N, n_cores, nch, npcp):
    """Degree-sorted dst-major plan.

    Returns (pos[R], Ks[nch], toff[nch], TOT, calls[nch],
             idx16 [n_cores,128,8*TOT] i16, mask [n_cores,128,TOT] f32).
    """
    R = n_cores * npcp
    bases = [round(q * (R - SPAN) / (W - 1)) for q in range(W)]

    src = np.concatenate([np.asarray(edge_index[0], np.int64), np.arange(N)])
    dst = np.concatenate([np.asarray(edge_index[1], np.int64), np.arange(N)])
    deg = np.bincount(dst, minlength=R)
    pos = np.empty(R, np.int64)
    for k in range(n_cores):
        ids = np.arange(k * npcp, (k + 1) * npcp)
        order = np.argsort(deg[ids], kind="stable")
        pos[ids[order]] = k * npcp + np.arange(npcp)
    srow = pos[src]
    dpos = pos[dst]
    key = (dpos // npcp * nch + (dpos % npcp) // 128) * 128 + dpos % 128
    order_e = np.lexsort((srow, key))
    ks, ss = key[order_e], srow[order_e]
    counts = np.bincount(ks, minlength=n_cores * nch * 128)
    maxd = int(counts.max())
    starts = np.zeros(len(counts) + 1, np.int64)
    np.cumsum(counts, out=starts[1:])
    col = np.arange(len(ss)) - starts[ks]
    Emat = np.full((n_cores * nch * 128, maxd), 2**31, np.int64)
    Emat[ks, col] = ss

    def plan_chunk(E, degv):
        valid = E < 2**31
        A = np.zeros(W, np.int64)
        B = np.zeros(W, np.int64)
        dmax = int(degv.max())
        for q in range(W - 1):
            A[q] = int(((E < bases[q + 1]) & valid).sum(axis=1).max())
            B[q] = int(((E >= bases[q] + SPAN) & valid).sum(axis=1).max())
        A[W - 1] = dmax
        K = int(max(dmax, (A + B).max(), 1))
        L = E.shape[0]
        while True:
            P = np.maximum.accumulate(np.minimum(np.maximum(A, 0), K - B))
            P[W - 1] = K
            n = np.diff(np.concatenate([[0], P]))
            qcls = np.repeat(np.arange(W), n)
            ptr = np.zeros(L, np.int64)
            slotidx = np.zeros((L, K), np.int32)
            slotmask = np.zeros((L, K), bool)
            ok = True
            for t in range(K):
                b = bases[qcls[t]]
                cur = E[np.arange(L), np.minimum(ptr, maxd - 1)]
                vv = ptr < degv
                if np.any(vv & (cur < b)):
                    ok = False
                    break
                fit = vv & (cur >= b) & (cur < b + SPAN)
                slotidx[:, t] = np.where(fit, cur - b, 0)
                slotmask[:, t] = fit
                ptr += fit
            if ok and np.all(ptr == degv):
                return K, qcls, slotidx, slotmask
            K += 1
            assert K < dmax + 24, "edge window planning failed to converge"

    Ks, toff, calls = [], [], []
    blocks_idx, blocks_mask = [], []
    off = 0
    for c in range(nch):
        lanes = ((np.arange(n_cores)[:, None] * nch + c) * 128
                 + np.arange(128)[None, :]).ravel()
        K, qcls, si, sm = plan_chunk(Emat[lanes], counts[lanes])
        Ks.append(K)
        toff.append(off)
        cc = []
        t0 = 0
        while t0 < K:
            q = qcls[t0]
            t1 = t0
            while t1 < K and qcls[t1] == q and t1 - t0 < GMAX:
                t1 += 1
            cc.append((t0, t1, int(q)))
            t0 = t1
        calls.append(cc)
        si = si.reshape(n_cores, 128, K)
        sm = sm.reshape(n_cores, 128, K)
        # idx layout: token T=off+t, partition p -> [p%16, 8*T + p//16]
        tmp = si.reshape(n_cores, 8, 16, K)          # p = s*16 + r
        blocks_idx.append(np.ascontiguousarray(
            tmp.transpose(0, 2, 3, 1)).reshape(n_cores, 16, 8 * K))
        blocks_mask.append(sm)
        off += K
    TOT = off
    idx16 = np.concatenate(blocks_idx, axis=2).astype(np.int16)
    idx16 = np.tile(idx16, (1, 8, 1))               # [n_cores, 128, 8*TOT]
    mask = np.concatenate(blocks_mask, axis=2).astype(np.float32)
    return pos, Ks, toff, TOT, calls, bases, idx16, mask


def _host_prep(x, edge_index, W1, att1_src, att1_dst, W2, att2_src, att2_dst):
    N, F = x.shape
    H, C = att1_src.shape
    HC = H * C
    NCLS = W2.shape[1]
    n_cores = N_CORES
    nch = -(-N // (n_cores * 128))
    npcp = nch * 128
    R = n_cores * npcp

    pos, Ks, toff, TOT, calls, bases, idx16, mask = _edge_plan(
        edge_index, N, n_cores, nch, npcp)

    # Folded attention-logit weight columns
    Wa_s = np.einsum("fhc,hc->fh", W1.reshape(F, H, C), att1_src).astype(np.float32)
    Wa_d = np.einsum("fhc,hc->fh", W1.reshape(F, H, C), att1_dst).astype(np.float32)
    W1e = np.ascontiguousarray(
        np.concatenate([W1, Wa_s, Wa_d], axis=1), dtype=np.float32)  # [F, 528]

    w2s = (W2 @ att2_src[0]).astype(np.float32)
    w2d = (W2 @ att2_dst[0]).astype(np.float32)
    W2e_flat = np.zeros((HC, 64), np.float32)
    W2e_flat[:, :NCLS] = W2
    W2e_flat[:, NCLS] = w2s
    W2e_flat[:, NCLS + 1] = w2d
    nslab = HC // 128
    W2e = np.ascontiguousarray(
        W2e_flat.reshape(nslab, 128, 64).transpose(1, 0, 2))  # [128, 4, 64]

    import ml_dtypes
    bf = ml_dtypes.bfloat16
    xtab = np.zeros((R, F), np.float32)
    xtab[pos[np.arange(N)]] = x
    xTp = np.ascontiguousarray(xtab.T).astype(bf)   # [F, R] permuted cols
    W1e = W1e.astype(bf)
    ident = np.eye(128, dtype=np.float32)

    cfg = dict(
        N=N, F=F, H=H, C=C, HC=HC, NCLS=NCLS, n_cores=n_cores,
        nch=nch, npcp=npcp, R=R, nslab=nslab,
        Ks=Ks, toff=toff, TOT=TOT, calls=calls, bases=bases, pos=pos,
        swdge_queues=4,
    )
    shared = dict(xTp=xTp, W1e=W1e, W2e=W2e, ident=ident)
    per_core = [
        dict(g1idx=idx16[k], mask=mask[k].astype(bf))
        for k in range(n_cores)
    ]
    return cfg, shared, per_core


# ----------------------------------------------------------------------------
# Device program
# ----------------------------------------------------------------------------

def _build_program(cfg):
    F, HC, NCLS = cfg["F"], cfg["HC"], cfg["NCLS"]
    n_cores, npcp, R = cfg["n_cores"], cfg["npcp"], cfg["R"]
    nslab, TOT = cfg["nslab"], cfg["TOT"]
    ROW1, ROW2 = 640, 128

    nc = bacc.Bacc("TRN2", target_bir_lowering=False, debug=False,
                   num_devices=n_cores,
                   num_swdge_queues=cfg.get("swdge_queues", 1))

    xTp = nc.dram_tensor("xTp", [F, R], BF16, kind="ExternalInput").ap()
    W1e = nc.dram_tensor("W1e", [F, HC + 16], BF16, kind="ExternalInput").ap()
    W2e = nc.dram_tensor("W2e", [128, nslab, 64], F32, kind="ExternalInput").ap()
    ident_d = nc.dram_tensor("ident", [128, 128], F32, kind="ExternalInput").ap()
    g1idx = nc.dram_tensor("g1idx", [128, 8 * TOT], I16,
                           kind="ExternalInput").ap()
    mask_d = nc.dram_tensor("mask", [128, TOT], BF16, kind="ExternalInput").ap()

    T1 = nc.dram_tensor("T1", [R, ROW1], BF16).ap()
    tb2_own = nc.dram_tensor("tb2_own", [npcp, ROW2], BF16).ap()
    tb2_full = nc.dram_tensor("tb2_full", [R, ROW2], BF16,
                              addr_space="Shared").ap()
    out2 = nc.dram_tensor("out2", [npcp, NCLS], F32, kind="ExternalOutput").ap()

    tensors = dict(xTp=xTp, W1e=W1e, W2e=W2e, ident=ident_d, g1idx=g1idx,
                   mask=mask_d, T1=T1, tb2_own=tb2_own, tb2_full=tb2_full,
                   out2=out2)
    repeat = cfg.get("repeat", 1)
    with tile.TileContext(nc) as tc:
        for _ in range(repeat):
            _emit(tc, cfg, tensors)
    nc.compile()
    return nc


def _emit(tc, cfg, t):
    nc = tc.nc
    H, HC, NCLS = cfg["H"], cfg["HC"], cfg["NCLS"]
    n_cores, nch, npcp, R = cfg["n_cores"], cfg["nch"], cfg["npcp"], cfg["R"]
    nslab = cfg["nslab"]
    NTB = R // 128

    with tc.tile_pool(name="consts", bufs=1) as cpool:
        W1e_sb = cpool.tile([128, HC + 16], BF16)
        nc.sync.dma_start(W1e_sb[:], t["W1e"][:, :])
        W2e_sb = cpool.tile([128, nslab, 64], F32)
        nc.sync.dma_start(W2e_sb[:], t["W2e"][:, :, :])
        identf_sb = cpool.tile([128, 128], F32)
        nc.sync.dma_start(identf_sb[:], t["ident"][:, :])
        ident_bf = cpool.tile([128, 128], BF16)
        nc.vector.tensor_copy(ident_bf[:], identf_sb[:])
        ald1_all = cpool.tile([128, NTB, H], F32)
        ald1_sb = cpool.tile([128, nch, H], F32)
        ald2_sb = cpool.tile([128, nch, 1], F32)

        # ---------------- Phase 0: permuted node table T1 ----------------
        with (
            tc.tile_pool(name="p0", bufs=3) as pool,
            tc.tile_pool(name="p0ps", bufs=3, space="PSUM") as pps,
        ):
            nblk = R // 512
            for i in range(nblk):
                xt = pool.tile([128, 512], BF16, tag="xt")
                nc.sync.dma_start(xt[:], t["xTp"][:, 512 * i: 512 * i + 512])
                rowB = pool.tile([128, 4, HC + 16], BF16, tag="rowB")
                for j in range(4):
                    ps = pps.tile([128, 1024], F32, tag="ps")
                    nc.tensor.matmul(ps[:, 0:HC], lhsT=xt[:, 128 * j: 128 * j + 128],
                                     rhs=W1e_sb[:, 0:HC], start=True, stop=True)
                    nc.tensor.matmul(ps[:, 512: 512 + 16],
                                     lhsT=xt[:, 128 * j: 128 * j + 128],
                                     rhs=W1e_sb[:, HC: HC + 16],
                                     start=True, stop=True)
                    nc.vector.tensor_copy(rowB[:, j, 0:HC], ps[:, 0:HC])
                    nc.scalar.copy(rowB[:, j, HC: HC + 16].bitcast(F32),
                                   ps[:, 512: 512 + H])
                    nc.scalar.copy(ald1_all[:, 4 * i + j, :],
                                   ps[:, 512 + H: 512 + 2 * H])
                nc.sync.dma_start(
                    t["T1"][512 * i: 512 * i + 512, 0: HC + 16].rearrange(
                        "(j p) c -> p j c", p=128),
                    rowB[:],
                )

        pid = nc.partition_id()
        nc.sync.dma_start(ald1_sb[:], ald1_all[:, bass.ds(pid * nch, nch), :])

        if cfg.get("phases", "full") == "p0":
            return
        # ---------------- L1 edge phase ----------------
        _edge_phase(tc, cfg, layer=1, gather_src=t["T1"], grow=640,
                    idx_d=t["g1idx"], mask_d=t["mask"],
                    ald_sb=ald1_sb, identf_sb=identf_sb, ident_bf=ident_bf,
                    W2e_sb=W2e_sb, tb2_own=t["tb2_own"], out2=None,
                    ald2_cap=ald2_sb)

        if cfg.get("phases", "full") == "p0+l1":
            return
        # ---------------- allgather ----------------
        if cfg.get("no_collective"):
            nc.sync.dma_start(t["tb2_full"][0:npcp, :], t["tb2_own"][:, :])
        else:
            nc.gpsimd.collective_compute(
                "AllGather",
                OP.bypass,
                replica_groups=[list(range(n_cores))],
                ins=[t["tb2_own"][:, :]],
                outs=[t["tb2_full"][:, :]],
            )

        if cfg.get("phases", "full") == "p0+l1+ag":
            return
        # ---------------- L2 edge phase ----------------
        _edge_phase(tc, cfg, layer=2, gather_src=t["tb2_full"], grow=128,
                    idx_d=t["g1idx"], mask_d=t["mask"],
                    ald_sb=ald2_sb, identf_sb=identf_sb, ident_bf=ident_bf,
                    W2e_sb=None, tb2_own=None, out2=t["out2"])


def _edge_phase(tc, cfg, layer, gather_src, grow, idx_d, mask_d, ald_sb,
                identf_sb, ident_bf, W2e_sb, tb2_own, out2, ald2_cap=None):
    nc = tc.nc
    nch, H, HC, NCLS = cfg["nch"], cfg["H"], cfg["HC"], cfg["NCLS"]
    nslab = cfg["nslab"]
    Ks, toff, calls, bases = cfg["Ks"], cfg["toff"], cfg["calls"], cfg["bases"]
    HL = H if layer == 1 else 1      # heads this layer
    als_off = HC if layer == 1 else NCLS  # bf16 col of al_src f32 pairs

    with (
        tc.tile_pool(name=f"gt{layer}", bufs=2) as gpool,
        tc.tile_pool(name=f"meta{layer}", bufs=3) as mpool,
        tc.tile_pool(name=f"msg{layer}", bufs=4) as msgpool,
        tc.tile_pool(name=f"small{layer}", bufs=3) as smpool,
        tc.tile_pool(name=f"out{layer}", bufs=2) as opool,
        tc.tile_pool(name=f"ps_u{layer}", bufs=2, space="PSUM") as pp_u,
        tc.tile_pool(name=f"ps_tr{layer}", bufs=2, space="PSUM") as pp_tr,
        tc.tile_pool(name=f"ps_o{layer}", bufs=2, space="PSUM") as pp_o,
    ):
        for c in range(nch):
            K = Ks[c]
            off = toff[c]
            gt = gpool.tile([128, K, grow], BF16, tag="gt")
            idx = mpool.tile([128, 8 * K], I16, tag="idx")
            nc.sync.dma_start(idx[:], idx_d[:, 8 * off: 8 * (off + K)])
            msk = mpool.tile([128, K], BF16, tag="msk")
            nc.sync.dma_start(msk[:], mask_d[:, off: off + K])
            nq = cfg.get("swdge_queues", 1)
            for (b0, b1, q) in calls[c]:
                nk = b1 - b0
                nc.gpsimd.dma_gather(
                    gt[:, b0:b1, :],
                    gather_src[bases[q]: bases[q] + SPAN, :],
                    idx[:, 8 * b0: 8 * b1],
                    nk * 128, nk * 128, grow,
                    queue_num=_QCTR[0] % nq,
                )
                _QCTR[0] += 1
            if layer == 1 and cfg.get("l1_mode") == "gather":
                continue

            # p = exp(leakyrelu(al_src[src] + al_dst[dst])) * mask
            s_t = smpool.tile([128, K, HL], F32, tag="s")
            nc.vector.tensor_tensor(
                s_t[:],
                gt[:, :, als_off: als_off + 2 * HL].bitcast(F32),
                ald_sb[:, c, None, :].to_broadcast([128, K, HL]),
                op=OP.add,
            )
            l_t = smpool.tile([128, K, HL], F32, tag="l")
            nc.vector.scalar_tensor_tensor(
                l_t[:], s_t[:], 0.2, s_t[:], op0=OP.mult, op1=OP.max
            )
            p_t = smpool.tile([128, K, HL], F32, tag="p")
            nc.scalar.activation(p_t[:], l_t[:], AF.Exp)
            p_bf = smpool.tile([128, K, HL], BF16, tag="pbf")
            nc.vector.tensor_tensor(
                p_bf[:], p_t[:],
                msk[:, :, None].to_broadcast([128, K, HL]),
                op=OP.mult,
            )

            if layer == 1:
                ps_u = pp_u.tile([128, HC], F32, tag="u")
                for k in range(K):
                    msg = msgpool.tile([128, HC], BF16, tag="msg")
                    nc.vector.tensor_tensor(
                        msg[:].rearrange("p (h c) -> p h c", h=H),
                        gt[:, k, 0:HC].rearrange("p (h c) -> p h c", h=H),
                        p_bf[:, k, :, None].to_broadcast([128, H, HC // H]),
                        op=OP.mult,
                    )
                    nc.tensor.matmul(
                        ps_u[:], lhsT=ident_bf[:], rhs=msg[:],
                        start=(k == 0), stop=(k == K - 1),
                    )
                zr = smpool.tile([128, H], F32, tag="zr")
                nc.vector.tensor_reduce(
                    zr[:], p_bf[:].rearrange("p k h -> p h k"),
                    axis=AX.X, op=OP.add,
                )
                zb = smpool.tile([128, H], F32, tag="zb")
                nc.vector.tensor_scalar_max(zb[:], zr[:], 1e-30)
                rz = smpool.tile([128, H], F32, tag="rz")
                nc.vector.reciprocal(rz[:], zb[:])
                h2 = opool.tile([128, HC], F32, tag="h2")
                nc.vector.tensor_tensor(
                    h2[:].rearrange("p (h c) -> p h c", h=H),
                    ps_u[:].rearrange("p (h c) -> p h c", h=H),
                    rz[:, :, None].to_broadcast([128, H, HC // H]),
                    op=OP.mult,
                )
                h2r = opool.tile([128, HC], F32, tag="h2r")
                nc.scalar.activation(h2r[:], h2[:], AF.Relu)
                ps_o = pp_o.tile([128, 64], F32, tag="o")
                for j in range(nslab):
                    ps_tr = pp_tr.tile([128, 128], F32, tag="tr")
                    nc.tensor.transpose(
                        ps_tr[:], h2r[:, 128 * j: 128 * (j + 1)], identf_sb[:]
                    )
                    h2t = smpool.tile([128, 128], F32, tag="h2t")
                    nc.scalar.copy(h2t[:], ps_tr[:])
                    nc.tensor.matmul(
                        ps_o[:], lhsT=h2t[:], rhs=W2e_sb[:, j, :],
                        start=(j == 0), stop=(j == nslab - 1),
                    )
                trow = opool.tile([128, 128], BF16, tag="trow")
                nc.vector.tensor_copy(trow[:, 0:NCLS], ps_o[:, 0:NCLS])
                nc.scalar.copy(trow[:, NCLS: NCLS + 4].bitcast(F32),
                               ps_o[:, NCLS: NCLS + 2])
                nc.scalar.copy(ald2_cap[:, c, :], ps_o[:, NCLS + 1: NCLS + 2])
                nc.sync.dma_start(tb2_own[128 * c: 128 * (c + 1), :], trow[:])
            else:
                msg2 = msgpool.tile([128, K, NCLS], BF16, tag="msg2")
                nc.vector.tensor_tensor(
                    msg2[:], gt[:, :, 0:NCLS],
                    p_bf[:, :, 0, None].to_broadcast([128, K, NCLS]),
                    op=OP.mult,
                )
                u2 = smpool.tile([128, NCLS], F32, tag="u2")
                nc.vector.tensor_reduce(
                    u2[:], msg2[:].rearrange("p k f -> p f k"),
                    axis=AX.X, op=OP.add,
                )
                z2 = smpool.tile([128, 1], F32, tag="z2")
                nc.vector.tensor_reduce(
                    z2[:], p_bf[:, :, 0], axis=AX.X, op=OP.add,
                )
                zb2 = smpool.tile([128, 1], F32, tag="zb2")
                nc.vector.tensor_scalar_max(zb2[:], z2[:], 1e-30)
                rz2 = smpool.tile([128, 1], F32, tag="rz2")
                nc.vector.reciprocal(rz2[:], zb2[:])
                o2 = opool.tile([128, NCLS], F32, tag="o2")
                nc.vector.tensor_tensor(
                    o2[:], u2[:], rz2[:].to_broadcast([128, NCLS]), op=OP.mult,
                )
                nc.sync.dma_start(out2[128 * c: 128 * (c + 1), :], o2[:])


# ----------------------------------------------------------------------------
# PJRT execution (with on-device iteration chaining for timing)
# ----------------------------------------------------------------------------

def _pjrt_exec(nc, in_maps, n_cores, iters=1, reps=3):
    import jax
    import numpy as _np
    from jax.sharding import Mesh, PartitionSpec
    from jax.experimental.shard_map import shard_map
    from concourse import bass2jax as b2j
    from concourse import mybir as _mb

    b2j.install_neuronx_cc_hook()
    partition_name = (nc.partition_id_tensor.name
                      if nc.partition_id_tensor else None)
    in_names, out_names, out_avals, zero_outs = [], [], [], []
    for alloc in nc.m.functions[0].allocations:
        if not isinstance(alloc, _mb.MemoryLocationSet):
            continue
        name = alloc.memorylocations[0].name
        if alloc.kind == "ExternalInput":
            if name != partition_name:
                in_names.append(name)
        elif alloc.kind == "ExternalOutput":
            shape = tuple(alloc.tensor_shape)
            dtype = _mb.dt.np(alloc.dtype)
            out_names.append(name)
            out_avals.append(jax.core.ShapedArray(shape, dtype))
            zero_outs.append(_np.zeros(shape, dtype))
    n_params = len(in_names)
    all_in_names = in_names + out_names
    if partition_name is not None:
        all_in_names = all_in_names + [partition_name]

    def _body(*args):
        ins = list(args[:n_params])
        zo = list(args[n_params:])
        for _ in range(iters):
            operands = ins + zo
            if partition_name is not None:
                operands.append(b2j.partition_id_tensor())
            outs = _bass_exec_bind(b2j, operands, out_avals, all_in_names,
                                   out_names, nc)
            zo = list(outs)
        return tuple(zo)

    devices = jax.devices()[:n_cores]
    mesh = Mesh(_np.asarray(devices), ("core",))
    in_specs = (PartitionSpec("core"),) * (n_params + len(out_names))
    out_specs = (PartitionSpec("core"),) * len(out_names)
    sharded = jax.jit(shard_map(_body, mesh=mesh, in_specs=in_specs,
                                out_specs=out_specs, check_rep=False),
                      keep_unused=True)
    concat_in = [
        _np.concatenate([_np.asarray(in_maps[c][nm]) for c in range(n_cores)],
                        axis=0)
        for nm in in_names
    ]
    concat_zeros = [_np.zeros((n_cores * z.shape[0], *z.shape[1:]), z.dtype)
                    for z in zero_outs]
    import time as _time
    from jax.sharding import NamedSharding
    sh = NamedSharding(mesh, PartitionSpec("core"))
    dev_in = [jax.device_put(a, sh) for a in concat_in]
    dev_zeros = [jax.device_put(a, sh) for a in concat_zeros]
    jax.block_until_ready(dev_in + dev_zeros)
    out_arrs = sharded(*dev_in, *dev_zeros)
    jax.block_until_ready(out_arrs)
    times = []
    for _ in range(reps):
        t0 = _time.perf_counter()
        out_arrs = sharded(*dev_in, *dev_zeros)
        jax.block_until_ready(out_arrs)
        times.append(_time.perf_counter() - t0)
    dt = min(times)
    results = [
        {nm: _np.asarray(out_arrs[i]).reshape(n_cores, *out_avals[i].shape)[c]
         for i, nm in enumerate(out_names)}
        for c in range(n_cores)
    ]
    return results, dt


def _bass_exec_bind(b2j, operands, out_avals, in_names, out_names, nc):
    return b2j._bass_exec_p.bind(
        *operands,
        out_avals=tuple(out_avals),
        in_names=tuple(in_names),
        out_names=tuple(out_names),
        lowering_input_output_aliases=(),
        sim_require_finite=True,
        sim_require_nnan=True,
        nc=nc,
    )


# ----------------------------------------------------------------------------
# Entry point
# ----------------------------------------------------------------------------

_CACHE = {}


def _run(inputs, trace=False):
    x = np.asarray(inputs["x"], np.float32)
    edge_index = np.asarray(inputs["edge_index"], np.int32)
    W1 = np.asarray(inputs["W1"], np.float32)
    a1s = np.asarray(inputs["att1_src"], np.float32)
    a1d = np.asarray(inputs["att1_dst"], np.float32)
    W2 = np.asarray(inputs["W2"], np.float32)
    a2s = np.asarray(inputs["att2_src"], np.float32)
    a2d = np.asarray(inputs["att2_dst"], np.float32)
    b1 = np.asarray(inputs["b1"], np.float32)
    b2 = np.asarray(inputs["b2"], np.float32)
    assert not b1.any() and not b2.any(), "nonzero bias unsupported"

    key = hashlib.sha1(
        b"v2" + edge_index.tobytes() + np.int64(x.shape).tobytes()
    ).hexdigest()
    cfg, shared, per_core = _host_prep(x, edge_index, W1, a1s, a1d, W2, a2s, a2d)
    if key not in _CACHE:
        _CACHE[key] = _build_program(cfg)
    nc = _CACHE[key]

    in_maps = []
    for k in range(cfg["n_cores"]):
        m = dict(shared)
        m.update(per_core[k])
        in_maps.append(m)
    res = run_bass_kernel_spmd(nc, in_maps, list(range(cfg["n_cores"])),
                               trace=trace)
    out = gather_out([res.results[k]["out2"] for k in range(cfg["n_cores"])],
                     cfg)
    return out.astype(np.float32), res


def gather_out(outs, cfg):
    allrows = np.concatenate(outs, axis=0)          # [R, NCLS] permuted
    return allrows[cfg["pos"][: cfg["N"]]]


def kernel(**inputs):
    out, _ = _run(inputs, trace=False)
    return out


# revision 10
# speedup vs baseline: 2.4253x; 2.4253x over previous
"""GAT (2-layer, PyG-default) Trainium2 Bass kernel, 8-core SPMD.

v2 — destination-major edge layout:
  - Nodes are permuted so each core's 6272 dst nodes are sorted by
    in-degree; the node table T1 is stored in this permuted order.  A
    chunk = 128 consecutive permuted dsts (uniform degree), one per
    SBUF partition.  Edges of dst p sit at [partition p, slot k] of the
    chunk's gather tile, so the edge->dst scatter matrix is the
    IDENTITY: aggregation is one accumulating PE matmul per 128-edge
    slot, and softmax (logits, leakyrelu, exp, masking, z) is pure
    elementwise DVE/ACT work.  No per-token transposes or selection
    matrices.
  - Phase 0 (replicated): T1[pos, :] = [h1(512) | al_src f32(8)] from
    x @ [W1 | W1@Asrc], batched 512 rows per DMA; al_dst kept on-chip.
  - int16 gather indices span only 32768 rows, so each slot is bound
    to one of W=4 overlapping 32768-row windows; a host-side greedy
    (Hall prefix/suffix sizing) assigns each dst's edges to slots.
    Self-loops are ordinary edges.  Pad slots gather window base row 0
    and are zeroed via a {0,1} mask multiplied into exp(logit).
  - L1 chunk result -> relu -> fused W2_ext projection -> tb2 row
    (40 cls | al2_src | al2_dst as f32 pairs); AllGather shares tables;
    L2 repeats with 256B rows and DVE-only aggregation (40 cols).

Self-contained: only needs numpy + the concourse tree at /opt/trn_rl_repo.
"""

import hashlib
import sys

import numpy as np

for _p in ("/opt/trn_rl_repo",):
    if _p not in sys.path:
        sys.path.insert(0, _p)

import concourse.bacc as bacc
import concourse.bass as bass
import concourse.tile as tile
from concourse import mybir
from concourse.bass_utils import run_bass_kernel_spmd

F32 = mybir.dt.float32
BF16 = mybir.dt.bfloat16
I16 = mybir.dt.int16
AF = mybir.ActivationFunctionType
OP = mybir.AluOpType
AX = mybir.AxisListType

N_CORES = 8
SPAN = 32768
W = 4
GMAX = 8
_QCTR = [0]  # global SWDGE queue round-robin


# ----------------------------------------------------------------------------
# Host-side edge planning
# ----------------------------------------------------------------------------

def _edge_plan(edge_index, N, n_cores, nch, npcp):
    """Degree-sorted dst-major plan.

    Returns (pos[R], Ks[nch], toff[nch], TOT, calls[nch],
             idx16 [n_cores,128,8*TOT] i16, mask [n_cores,128,TOT] f32).
    """
    R = n_cores * npcp
    bases = [round(q * (R - SPAN) / (W - 1)) for q in range(W)]

    src = np.concatenate([np.asarray(edge_index[0], np.int64), np.arange(N)])
    dst = np.concatenate([np.asarray(edge_index[1], np.int64), np.arange(N)])
    deg = np.bincount(dst, minlength=R)
    pos = np.empty(R, np.int64)
    for k in range(n_cores):
        ids = np.arange(k * npcp, (k + 1) * npcp)
        order = np.argsort(deg[ids], kind="stable")
        pos[ids[order]] = k * npcp + np.arange(npcp)
    srow = pos[src]
    dpos = pos[dst]
    key = (dpos // npcp * nch + (dpos % npcp) // 128) * 128 + dpos % 128
    order_e = np.lexsort((srow, key))
    ks, ss = key[order_e], srow[order_e]
    counts = np.bincount(ks, minlength=n_cores * nch * 128)
    maxd = int(counts.max())
    starts = np.zeros(len(counts) + 1, np.int64)
    np.cumsum(counts, out=starts[1:])
    col = np.arange(len(ss)) - starts[ks]
    Emat = np.full((n_cores * nch * 128, maxd), 2**31, np.int64)
    Emat[ks, col] = ss

    def plan_chunk(E, degv):
        valid = E < 2**31
        A = np.zeros(W, np.int64)
        B = np.zeros(W, np.int64)
        dmax = int(degv.max())
        for q in range(W - 1):
            A[q] = int(((E < bases[q + 1]) & valid).sum(axis=1).max())
            B[q] = int(((E >= bases[q] + SPAN) & valid).sum(axis=1).max())
        A[W - 1] = dmax
        K = int(max(dmax, (A + B).max(), 1))
        L = E.shape[0]
        while True:
            P = np.maximum.accumulate(np.minimum(np.maximum(A, 0), K - B))
            P[W - 1] = K
            n = np.diff(np.concatenate([[0], P]))
            qcls = np.repeat(np.arange(W), n)
            ptr = np.zeros(L, np.int64)
            slotidx = np.zeros((L, K), np.int32)
            slotmask = np.zeros((L, K), bool)
            ok = True
            for t in range(K):
                b = bases[qcls[t]]
                cur = E[np.arange(L), np.minimum(ptr, maxd - 1)]
                vv = ptr < degv
                if np.any(vv & (cur < b)):
                    ok = False
                    break
                fit = vv & (cur >= b) & (cur < b + SPAN)
                slotidx[:, t] = np.where(fit, cur - b, 0)
                slotmask[:, t] = fit
                ptr += fit
            if ok and np.all(ptr == degv):
                return K, qcls, slotidx, slotmask
            K += 1
            assert K < dmax + 24, "edge window planning failed to converge"

    Ks, toff, calls = [], [], []
    blocks_idx, blocks_mask = [], []
    off = 0
    for c in range(nch):
        lanes = ((np.arange(n_cores)[:, None] * nch + c) * 128
                 + np.arange(128)[None, :]).ravel()
        K, qcls, si, sm = plan_chunk(Emat[lanes], counts[lanes])
        Ks.append(K)
        toff.append(off)
        cc = []
        t0 = 0
        while t0 < K:
            q = qcls[t0]
            t1 = t0
            while t1 < K and qcls[t1] == q and t1 - t0 < GMAX:
                t1 += 1
            cc.append((t0, t1, int(q)))
            t0 = t1
        calls.append(cc)
        si = si.reshape(n_cores, 128, K)
        sm = sm.reshape(n_cores, 128, K)
        # idx layout: token T=off+t, partition p -> [p%16, 8*T + p//16]
        tmp = si.reshape(n_cores, 8, 16, K)          # p = s*16 + r
        blocks_idx.append(np.ascontiguousarray(
            tmp.transpose(0, 2, 3, 1)).reshape(n_cores, 16, 8 * K))
        blocks_mask.append(sm)
        off += K
    TOT = off
    idx16 = np.concatenate(blocks_idx, axis=2).astype(np.int16)
    idx16 = np.tile(idx16, (1, 8, 1))               # [n_cores, 128, 8*TOT]
    mask = np.concatenate(blocks_mask, axis=2).astype(np.float32)
    return pos, Ks, toff, TOT, calls, bases, idx16, mask


def _host_prep(x, edge_index, W1, att1_src, att1_dst, W2, att2_src, att2_dst):
    N, F = x.shape
    H, C = att1_src.shape
    HC = H * C
    NCLS = W2.shape[1]
    n_cores = N_CORES
    nch = -(-N // (n_cores * 128))
    npcp = nch * 128
    R = n_cores * npcp

    pos, Ks, toff, TOT, calls, bases, idx16, mask = _edge_plan(
        edge_index, N, n_cores, nch, npcp)

    # Folded attention-logit weight columns
    Wa_s = np.einsum("fhc,hc->fh", W1.reshape(F, H, C), att1_src).astype(np.float32)
    Wa_d = np.einsum("fhc,hc->fh", W1.reshape(F, H, C), att1_dst).astype(np.float32)
    W1e = np.ascontiguousarray(
        np.concatenate([W1, Wa_s, Wa_d], axis=1), dtype=np.float32)  # [F, 528]

    w2s = (W2 @ att2_src[0]).astype(np.float32)
    w2d = (W2 @ att2_dst[0]).astype(np.float32)
    W2e_flat = np.zeros((HC, 64), np.float32)
    W2e_flat[:, :NCLS] = W2
    W2e_flat[:, NCLS] = w2s
    W2e_flat[:, NCLS + 1] = w2d
    nslab = HC // 128
    W2e = np.ascontiguousarray(
        W2e_flat.reshape(nslab, 128, 64).transpose(1, 0, 2))  # [128, 4, 64]

    import ml_dtypes
    bf = ml_dtypes.bfloat16
    xtab = np.zeros((R, F), np.float32)
    xtab[pos[np.arange(N)]] = x
    xTp = np.ascontiguousarray(xtab.T).astype(bf)   # [F, R] permuted cols
    W1e = W1e.astype(bf)
    ident = np.eye(128, dtype=np.float32)

    cfg = dict(
        N=N, F=F, H=H, C=C, HC=HC, NCLS=NCLS, n_cores=n_cores,
        nch=nch, npcp=npcp, R=R, nslab=nslab,
        Ks=Ks, toff=toff, TOT=TOT, calls=calls, bases=bases, pos=pos,
        swdge_queues=4, p0_bufs=4, gt_bufs=3,
    )
    shared = dict(xTp=xTp, W1e=W1e, W2e=W2e, ident=ident)
    per_core = [
        dict(g1idx=idx16[k], mask=mask[k].astype(bf))
        for k in range(n_cores)
    ]
    return cfg, shared, per_core


# ----------------------------------------------------------------------------
# Device program
# ----------------------------------------------------------------------------

def _build_program(cfg):
    F, HC, NCLS = cfg["F"], cfg["HC"], cfg["NCLS"]
    n_cores, npcp, R = cfg["n_cores"], cfg["npcp"], cfg["R"]
    nslab, TOT = cfg["nslab"], cfg["TOT"]
    ROW1, ROW2 = 640, 128

    nc = bacc.Bacc("TRN2", target_bir_lowering=False, debug=False,
                   num_devices=n_cores,
                   num_swdge_queues=cfg.get("swdge_queues", 1))

    xTp = nc.dram_tensor("xTp", [F, R], BF16, kind="ExternalInput").ap()
    W1e = nc.dram_tensor("W1e", [F, HC + 16], BF16, kind="ExternalInput").ap()
    W2e = nc.dram_tensor("W2e", [128, nslab, 64], F32, kind="ExternalInput").ap()
    ident_d = nc.dram_tensor("ident", [128, 128], F32, kind="ExternalInput").ap()
    g1idx = nc.dram_tensor("g1idx", [128, 8 * TOT], I16,
                           kind="ExternalInput").ap()
    mask_d = nc.dram_tensor("mask", [128, TOT], BF16, kind="ExternalInput").ap()

    T1 = nc.dram_tensor("T1", [R, ROW1], BF16).ap()
    tb2_own = nc.dram_tensor("tb2_own", [npcp, ROW2], BF16).ap()
    tb2_full = nc.dram_tensor("tb2_full", [R, ROW2], BF16,
                              addr_space="Shared").ap()
    out2 = nc.dram_tensor("out2", [npcp, NCLS], F32, kind="ExternalOutput").ap()

    tensors = dict(xTp=xTp, W1e=W1e, W2e=W2e, ident=ident_d, g1idx=g1idx,
                   mask=mask_d, T1=T1, tb2_own=tb2_own, tb2_full=tb2_full,
                   out2=out2)
    repeat = cfg.get("repeat", 1)
    with tile.TileContext(nc) as tc:
        for _ in range(repeat):
            _emit(tc, cfg, tensors)
    nc.compile()
    return nc


def _emit(tc, cfg, t):
    nc = tc.nc
    H, HC, NCLS = cfg["H"], cfg["HC"], cfg["NCLS"]
    n_cores, nch, npcp, R = cfg["n_cores"], cfg["nch"], cfg["npcp"], cfg["R"]
    nslab = cfg["nslab"]
    NTB = R // 128

    with tc.tile_pool(name="consts", bufs=1) as cpool:
        W1e_sb = cpool.tile([128, HC + 16], BF16)
        nc.sync.dma_start(W1e_sb[:], t["W1e"][:, :])
        W2e_sb = cpool.tile([128, nslab, 64], F32)
        nc.sync.dma_start(W2e_sb[:], t["W2e"][:, :, :])
        identf_sb = cpool.tile([128, 128], F32)
        nc.sync.dma_start(identf_sb[:], t["ident"][:, :])
        ident_bf = cpool.tile([128, 128], BF16)
        nc.vector.tensor_copy(ident_bf[:], identf_sb[:])
        ald1_all = cpool.tile([128, NTB, H], F32)
        ald1_sb = cpool.tile([128, nch, H], F32)
        ald2_sb = cpool.tile([128, nch, 1], F32)

        # ---------------- Phase 0: permuted node table T1 ----------------
        with (
            tc.tile_pool(name="p0", bufs=cfg.get("p0_bufs", 3)) as pool,
            tc.tile_pool(name="p0ps", bufs=cfg.get("p0_bufs", 3),
                         space="PSUM") as pps,
        ):
            nblk = R // 512
            for i in range(nblk):
                xt = pool.tile([128, 512], BF16, tag="xt")
                nc.sync.dma_start(xt[:], t["xTp"][:, 512 * i: 512 * i + 512])
                rowB = pool.tile([128, 4, HC + 16], BF16, tag="rowB")
                for j in range(4):
                    ps = pps.tile([128, 1024], F32, tag="ps")
                    nc.tensor.matmul(ps[:, 0:HC], lhsT=xt[:, 128 * j: 128 * j + 128],
                                     rhs=W1e_sb[:, 0:HC], start=True, stop=True)
                    nc.tensor.matmul(ps[:, 512: 512 + 16],
                                     lhsT=xt[:, 128 * j: 128 * j + 128],
                                     rhs=W1e_sb[:, HC: HC + 16],
                                     start=True, stop=True)
                    nc.vector.tensor_copy(rowB[:, j, 0:HC], ps[:, 0:HC])
                    nc.scalar.copy(rowB[:, j, HC: HC + 16].bitcast(F32),
                                   ps[:, 512: 512 + H])
                    nc.scalar.copy(ald1_all[:, 4 * i + j, :],
                                   ps[:, 512 + H: 512 + 2 * H])
                nc.sync.dma_start(
                    t["T1"][512 * i: 512 * i + 512, 0: HC + 16].rearrange(
                        "(j p) c -> p j c", p=128),
                    rowB[:],
                )

        pid = nc.partition_id()
        nc.sync.dma_start(ald1_sb[:], ald1_all[:, bass.ds(pid * nch, nch), :])

        if cfg.get("phases", "full") == "p0":
            return
        # ---------------- L1 edge phase ----------------
        _edge_phase(tc, cfg, layer=1, gather_src=t["T1"], grow=640,
                    idx_d=t["g1idx"], mask_d=t["mask"],
                    ald_sb=ald1_sb, identf_sb=identf_sb, ident_bf=ident_bf,
                    W2e_sb=W2e_sb, tb2_own=t["tb2_own"], out2=None,
                    ald2_cap=ald2_sb)

        if cfg.get("phases", "full") == "p0+l1":
            return
        # ---------------- allgather ----------------
        if cfg.get("no_collective"):
            nc.sync.dma_start(t["tb2_full"][0:npcp, :], t["tb2_own"][:, :])
        else:
            nc.gpsimd.collective_compute(
                "AllGather",
                OP.bypass,
                replica_groups=[list(range(n_cores))],
                ins=[t["tb2_own"][:, :]],
                outs=[t["tb2_full"][:, :]],
            )

        if cfg.get("phases", "full") == "p0+l1+ag":
            return
        # ---------------- L2 edge phase ----------------
        _edge_phase(tc, cfg, layer=2, gather_src=t["tb2_full"], grow=128,
                    idx_d=t["g1idx"], mask_d=t["mask"],
                    ald_sb=ald2_sb, identf_sb=identf_sb, ident_bf=ident_bf,
                    W2e_sb=None, tb2_own=None, out2=t["out2"])


def _edge_phase(tc, cfg, layer, gather_src, grow, idx_d, mask_d, ald_sb,
                identf_sb, ident_bf, W2e_sb, tb2_own, out2, ald2_cap=None):
    nc = tc.nc
    nch, H, HC, NCLS = cfg["nch"], cfg["H"], cfg["HC"], cfg["NCLS"]
    nslab = cfg["nslab"]
    Ks, toff, calls, bases = cfg["Ks"], cfg["toff"], cfg["calls"], cfg["bases"]
    HL = H if layer == 1 else 1      # heads this layer
    als_off = HC if layer == 1 else NCLS  # bf16 col of al_src f32 pairs

    with (
        tc.tile_pool(name=f"gt{layer}", bufs=cfg.get("gt_bufs", 2)) as gpool,
        tc.tile_pool(name=f"meta{layer}", bufs=3) as mpool,
        tc.tile_pool(name=f"msg{layer}", bufs=4) as msgpool,
        tc.tile_pool(name=f"small{layer}", bufs=3) as smpool,
        tc.tile_pool(name=f"out{layer}", bufs=2) as opool,
        tc.tile_pool(name=f"ps_u{layer}", bufs=2, space="PSUM") as pp_u,
        tc.tile_pool(name=f"ps_tr{layer}", bufs=2, space="PSUM") as pp_tr,
        tc.tile_pool(name=f"ps_o{layer}", bufs=2, space="PSUM") as pp_o,
    ):
        for c in range(nch):
            K = Ks[c]
            off = toff[c]
            gt = gpool.tile([128, K, grow], BF16, tag="gt")
            idx = mpool.tile([128, 8 * K], I16, tag="idx")
            nc.sync.dma_start(idx[:], idx_d[:, 8 * off: 8 * (off + K)])
            msk = mpool.tile([128, K], BF16, tag="msk")
            nc.sync.dma_start(msk[:], mask_d[:, off: off + K])
            nq = cfg.get("swdge_queues", 1)
            for (b0, b1, q) in calls[c]:
                nk = b1 - b0
                nc.gpsimd.dma_gather(
                    gt[:, b0:b1, :],
                    gather_src[bases[q]: bases[q] + SPAN, :],
                    idx[:, 8 * b0: 8 * b1],
                    nk * 128, nk * 128, grow,
                    queue_num=_QCTR[0] % nq,
                )
                _QCTR[0] += 1
            if layer == 1 and cfg.get("l1_mode") == "gather":
                continue

            # p = exp(leakyrelu(al_src[src] + al_dst[dst])) * mask
            s_t = smpool.tile([128, K, HL], F32, tag="s")
            nc.vector.tensor_tensor(
                s_t[:],
                gt[:, :, als_off: als_off + 2 * HL].bitcast(F32),
                ald_sb[:, c, None, :].to_broadcast([128, K, HL]),
                op=OP.add,
            )
            l_t = smpool.tile([128, K, HL], F32, tag="l")
            nc.vector.scalar_tensor_tensor(
                l_t[:], s_t[:], 0.2, s_t[:], op0=OP.mult, op1=OP.max
            )
            p_t = smpool.tile([128, K, HL], F32, tag="p")
            nc.scalar.activation(p_t[:], l_t[:], AF.Exp)
            p_bf = smpool.tile([128, K, HL], BF16, tag="pbf")
            nc.vector.tensor_tensor(
                p_bf[:], p_t[:],
                msk[:, :, None].to_broadcast([128, K, HL]),
                op=OP.mult,
            )

            if layer == 1:
                ps_u = pp_u.tile([128, HC], F32, tag="u")
                for k in range(K):
                    msg = msgpool.tile([128, HC], BF16, tag="msg")
                    nc.vector.tensor_tensor(
                        msg[:].rearrange("p (h c) -> p h c", h=H),
                        gt[:, k, 0:HC].rearrange("p (h c) -> p h c", h=H),
                        p_bf[:, k, :, None].to_broadcast([128, H, HC // H]),
                        op=OP.mult,
                    )
                    nc.tensor.matmul(
                        ps_u[:], lhsT=ident_bf[:], rhs=msg[:],
                        start=(k == 0), stop=(k == K - 1),
                    )
                zr = smpool.tile([128, H], F32, tag="zr")
                nc.vector.tensor_reduce(
                    zr[:], p_bf[:].rearrange("p k h -> p h k"),
                    axis=AX.X, op=OP.add,
                )
                zb = smpool.tile([128, H], F32, tag="zb")
                nc.vector.tensor_scalar_max(zb[:], zr[:], 1e-30)
                rz = smpool.tile([128, H], F32, tag="rz")
                nc.vector.reciprocal(rz[:], zb[:])
                h2 = opool.tile([128, HC], F32, tag="h2")
                nc.vector.tensor_tensor(
                    h2[:].rearrange("p (h c) -> p h c", h=H),
                    ps_u[:].rearrange("p (h c) -> p h c", h=H),
                    rz[:, :, None].to_broadcast([128, H, HC // H]),
                    op=OP.mult,
                )
                h2r = opool.tile([128, HC], F32, tag="h2r")
                nc.scalar.activation(h2r[:], h2[:], AF.Relu)
                ps_o = pp_o.tile([128, 64], F32, tag="o")
                for j in range(nslab):
                    ps_tr = pp_tr.tile([128, 128], F32, tag="tr")
                    nc.tensor.transpose(
                        ps_tr[:], h2r[:, 128 * j: 128 * (j + 1)], identf_sb[:]
                    )
                    h2t = smpool.tile([128, 128], F32, tag="h2t")
                    nc.scalar.copy(h2t[:], ps_tr[:])
                    nc.tensor.matmul(
                        ps_o[:], lhsT=h2t[:], rhs=W2e_sb[:, j, :],
                        start=(j == 0), stop=(j == nslab - 1),
                    )
                trow = opool.tile([128, 128], BF16, tag="trow")
                nc.vector.tensor_copy(trow[:, 0:NCLS], ps_o[:, 0:NCLS])
                nc.scalar.copy(trow[:, NCLS: NCLS + 4].bitcast(F32),
                               ps_o[:, NCLS: NCLS + 2])
                nc.scalar.copy(ald2_cap[:, c, :], ps_o[:, NCLS + 1: NCLS + 2])
                nc.sync.dma_start(tb2_own[128 * c: 128 * (c + 1), :], trow[:])
            else:
                msg2 = msgpool.tile([128, K, NCLS], BF16, tag="msg2")
                nc.vector.tensor_tensor(
                    msg2[:], gt[:, :, 0:NCLS],
                    p_bf[:, :, 0, None].to_broadcast([128, K, NCLS]),
                    op=OP.mult,
                )
                u2 = smpool.tile([128, NCLS], F32, tag="u2")
                nc.vector.tensor_reduce(
                    u2[:], msg2[:].rearrange("p k f -> p f k"),
                    axis=AX.X, op=OP.add,
                )
                z2 = smpool.tile([128, 1], F32, tag="z2")
                nc.vector.tensor_reduce(
                    z2[:], p_bf[:, :, 0], axis=AX.X, op=OP.add,
                )
                zb2 = smpool.tile([128, 1], F32, tag="zb2")
                nc.vector.tensor_scalar_max(zb2[:], z2[:], 1e-30)
                rz2 = smpool.tile([128, 1], F32, tag="rz2")
                nc.vector.reciprocal(rz2[:], zb2[:])
                o2 = opool.tile([128, NCLS], F32, tag="o2")
                nc.vector.tensor_tensor(
                    o2[:], u2[:], rz2[:].to_broadcast([128, NCLS]), op=OP.mult,
                )
                nc.sync.dma_start(out2[128 * c: 128 * (c + 1), :], o2[:])


# ----------------------------------------------------------------------------
# PJRT execution (with on-device iteration chaining for timing)
# ----------------------------------------------------------------------------

def _pjrt_exec(nc, in_maps, n_cores, iters=1, reps=3):
    import jax
    import numpy as _np
    from jax.sharding import Mesh, PartitionSpec
    from jax.experimental.shard_map import shard_map
    from concourse import bass2jax as b2j
    from concourse import mybir as _mb

    b2j.install_neuronx_cc_hook()
    partition_name = (nc.partition_id_tensor.name
                      if nc.partition_id_tensor else None)
    in_names, out_names, out_avals, zero_outs = [], [], [], []
    for alloc in nc.m.functions[0].allocations:
        if not isinstance(alloc, _mb.MemoryLocationSet):
            continue
        name = alloc.memorylocations[0].name
        if alloc.kind == "ExternalInput":
            if name != partition_name:
                in_names.append(name)
        elif alloc.kind == "ExternalOutput":
            shape = tuple(alloc.tensor_shape)
            dtype = _mb.dt.np(alloc.dtype)
            out_names.append(name)
            out_avals.append(jax.core.ShapedArray(shape, dtype))
            zero_outs.append(_np.zeros(shape, dtype))
    n_params = len(in_names)
    all_in_names = in_names + out_names
    if partition_name is not None:
        all_in_names = all_in_names + [partition_name]

    def _body(*args):
        ins = list(args[:n_params])
        zo = list(args[n_params:])
        for _ in range(iters):
            operands = ins + zo
            if partition_name is not None:
                operands.append(b2j.partition_id_tensor())
            outs = _bass_exec_bind(b2j, operands, out_avals, all_in_names,
                                   out_names, nc)
            zo = list(outs)
        return tuple(zo)

    devices = jax.devices()[:n_cores]
    mesh = Mesh(_np.asarray(devices), ("core",))
    in_specs = (PartitionSpec("core"),) * (n_params + len(out_names))
    out_specs = (PartitionSpec("core"),) * len(out_names)
    sharded = jax.jit(shard_map(_body, mesh=mesh, in_specs=in_specs,
                                out_specs=out_specs, check_rep=False),
                      keep_unused=True)
    concat_in = [
        _np.concatenate([_np.asarray(in_maps[c][nm]) for c in range(n_cores)],
                        axis=0)
        for nm in in_names
    ]
    concat_zeros = [_np.zeros((n_cores * z.shape[0], *z.shape[1:]), z.dtype)
                    for z in zero_outs]
    import time as _time
    from jax.sharding import NamedSharding
    sh = NamedSharding(mesh, PartitionSpec("core"))
    dev_in = [jax.device_put(a, sh) for a in concat_in]
    dev_zeros = [jax.device_put(a, sh) for a in concat_zeros]
    jax.block_until_ready(dev_in + dev_zeros)
    out_arrs = sharded(*dev_in, *dev_zeros)
    jax.block_until_ready(out_arrs)
    times = []
    for _ in range(reps):
        t0 = _time.perf_counter()
        out_arrs = sharded(*dev_in, *dev_zeros)
        jax.block_until_ready(out_arrs)
        times.append(_time.perf_counter() - t0)
    dt = min(times)
    results = [
        {nm: _np.asarray(out_arrs[i]).reshape(n_cores, *out_avals[i].shape)[c]
         for i, nm in enumerate(out_names)}
        for c in range(n_cores)
    ]
    return results, dt


def _bass_exec_bind(b2j, operands, out_avals, in_names, out_names, nc):
    return b2j._bass_exec_p.bind(
        *operands,
        out_avals=tuple(out_avals),
        in_names=tuple(in_names),
        out_names=tuple(out_names),
        lowering_input_output_aliases=(),
        sim_require_finite=True,
        sim_require_nnan=True,
        nc=nc,
    )


# ----------------------------------------------------------------------------
# Entry point
# ----------------------------------------------------------------------------

_CACHE = {}


def _run(inputs, trace=False):
    x = np.asarray(inputs["x"], np.float32)
    edge_index = np.asarray(inputs["edge_index"], np.int32)
    W1 = np.asarray(inputs["W1"], np.float32)
    a1s = np.asarray(inputs["att1_src"], np.float32)
    a1d = np.asarray(inputs["att1_dst"], np.float32)
    W2 = np.asarray(inputs["W2"], np.float32)
    a2s = np.asarray(inputs["att2_src"], np.float32)
    a2d = np.asarray(inputs["att2_dst"], np.float32)
    b1 = np.asarray(inputs["b1"], np.float32)
    b2 = np.asarray(inputs["b2"], np.float32)
    assert not b1.any() and not b2.any(), "nonzero bias unsupported"

    key = hashlib.sha1(
        b"v2" + edge_index.tobytes() + np.int64(x.shape).tobytes()
    ).hexdigest()
    cfg, shared, per_core = _host_prep(x, edge_index, W1, a1s, a1d, W2, a2s, a2d)
    if key not in _CACHE:
        _CACHE[key] = _build_program(cfg)
    nc = _CACHE[key]

    in_maps = []
    for k in range(cfg["n_cores"]):
        m = dict(shared)
        m.update(per_core[k])
        in_maps.append(m)
    res = run_bass_kernel_spmd(nc, in_maps, list(range(cfg["n_cores"])),
                               trace=trace)
    out = gather_out([res.results[k]["out2"] for k in range(cfg["n_cores"])],
                     cfg)
    return out.astype(np.float32), res


def gather_out(outs, cfg):
    allrows = np.concatenate(outs, axis=0)          # [R, NCLS] permuted
    return allrows[cfg["pos"][: cfg["N"]]]


def kernel(**inputs):
    out, _ = _run(inputs, trace=False)
    return out


# revision 13
# speedup vs baseline: 2.4547x; 1.0122x over previous
"""GAT (2-layer, PyG-default) Trainium2 Bass kernel, 8-core SPMD.

v2 — destination-major edge layout:
  - Nodes are permuted so each core's 6272 dst nodes are sorted by
    in-degree; the node table T1 is stored in this permuted order.  A
    chunk = 128 consecutive permuted dsts (uniform degree), one per
    SBUF partition.  Edges of dst p sit at [partition p, slot k] of the
    chunk's gather tile, so the edge->dst scatter matrix is the
    IDENTITY: aggregation is one accumulating PE matmul per 128-edge
    slot, and softmax (logits, leakyrelu, exp, masking, z) is pure
    elementwise DVE/ACT work.  No per-token transposes or selection
    matrices.
  - Phase 0 (replicated): T1[pos, :] = [h1(512) | al_src f32(8)] from
    x @ [W1 | W1@Asrc], batched 512 rows per DMA; al_dst kept on-chip.
  - int16 gather indices span only 32768 rows, so each slot is bound
    to one of W=4 overlapping 32768-row windows; a host-side greedy
    (Hall prefix/suffix sizing) assigns each dst's edges to slots.
    Self-loops are ordinary edges.  Pad slots gather window base row 0
    and are zeroed via a {0,1} mask multiplied into exp(logit).
  - L1 chunk result -> relu -> fused W2_ext projection -> tb2 row
    (40 cls | al2_src | al2_dst as f32 pairs); AllGather shares tables;
    L2 repeats with 256B rows and DVE-only aggregation (40 cols).

Self-contained: only needs numpy + the concourse tree at /opt/trn_rl_repo.
"""

import hashlib
import sys

import numpy as np

for _p in ("/opt/trn_rl_repo",):
    if _p not in sys.path:
        sys.path.insert(0, _p)

import concourse.bacc as bacc
import concourse.bass as bass
import concourse.tile as tile
from concourse import mybir
from concourse.bass_utils import run_bass_kernel_spmd

F32 = mybir.dt.float32
BF16 = mybir.dt.bfloat16
I16 = mybir.dt.int16
AF = mybir.ActivationFunctionType
OP = mybir.AluOpType
AX = mybir.AxisListType

N_CORES = 8
SPAN = 32768
W = 4
GMAX = 8
_QCTR = [0]  # global SWDGE queue round-robin


# ----------------------------------------------------------------------------
# Host-side edge planning
# ----------------------------------------------------------------------------

def _edge_plan(edge_index, N, n_cores, nch, npcp):
    """Degree-sorted dst-major plan.

    Returns (pos[R], Ks[nch], toff[nch], TOT, calls[nch],
             idx16 [n_cores,128,8*TOT] i16, mask [n_cores,128,TOT] f32).
    """
    R = n_cores * npcp
    bases = [round(q * (R - SPAN) / (W - 1)) for q in range(W)]

    src = np.concatenate([np.asarray(edge_index[0], np.int64), np.arange(N)])
    dst = np.concatenate([np.asarray(edge_index[1], np.int64), np.arange(N)])
    deg = np.bincount(dst, minlength=R)
    pos = np.empty(R, np.int64)
    for k in range(n_cores):
        ids = np.arange(k * npcp, (k + 1) * npcp)
        order = np.argsort(deg[ids], kind="stable")
        pos[ids[order]] = k * npcp + np.arange(npcp)
    srow = pos[src]
    dpos = pos[dst]
    key = (dpos // npcp * nch + (dpos % npcp) // 128) * 128 + dpos % 128
    order_e = np.lexsort((srow, key))
    ks, ss = key[order_e], srow[order_e]
    counts = np.bincount(ks, minlength=n_cores * nch * 128)
    maxd = int(counts.max())
    starts = np.zeros(len(counts) + 1, np.int64)
    np.cumsum(counts, out=starts[1:])
    col = np.arange(len(ss)) - starts[ks]
    Emat = np.full((n_cores * nch * 128, maxd), 2**31, np.int64)
    Emat[ks, col] = ss

    def plan_chunk(E, degv):
        valid = E < 2**31
        A = np.zeros(W, np.int64)
        B = np.zeros(W, np.int64)
        dmax = int(degv.max())
        for q in range(W - 1):
            A[q] = int(((E < bases[q + 1]) & valid).sum(axis=1).max())
            B[q] = int(((E >= bases[q] + SPAN) & valid).sum(axis=1).max())
        A[W - 1] = dmax
        K = int(max(dmax, (A + B).max(), 1))
        L = E.shape[0]
        while True:
            P = np.maximum.accumulate(np.minimum(np.maximum(A, 0), K - B))
            P[W - 1] = K
            n = np.diff(np.concatenate([[0], P]))
            qcls = np.repeat(np.arange(W), n)
            ptr = np.zeros(L, np.int64)
            slotidx = np.zeros((L, K), np.int32)
            slotmask = np.zeros((L, K), bool)
            ok = True
            for t in range(K):
                b = bases[qcls[t]]
                cur = E[np.arange(L), np.minimum(ptr, maxd - 1)]
                vv = ptr < degv
                if np.any(vv & (cur < b)):
                    ok = False
                    break
                fit = vv & (cur >= b) & (cur < b + SPAN)
                slotidx[:, t] = np.where(fit, cur - b, 0)
                slotmask[:, t] = fit
                ptr += fit
            if ok and np.all(ptr == degv):
                return K, qcls, slotidx, slotmask
            K += 1
            assert K < dmax + 24, "edge window planning failed to converge"

    Ks, toff, calls = [], [], []
    blocks_idx, blocks_mask = [], []
    off = 0
    for c in range(nch):
        lanes = ((np.arange(n_cores)[:, None] * nch + c) * 128
                 + np.arange(128)[None, :]).ravel()
        K, qcls, si, sm = plan_chunk(Emat[lanes], counts[lanes])
        Ks.append(K)
        toff.append(off)
        cc = []
        t0 = 0
        while t0 < K:
            q = qcls[t0]
            t1 = t0
            while t1 < K and qcls[t1] == q and t1 - t0 < GMAX:
                t1 += 1
            cc.append((t0, t1, int(q)))
            t0 = t1
        calls.append(cc)
        si = si.reshape(n_cores, 128, K)
        sm = sm.reshape(n_cores, 128, K)
        # idx layout: token T=off+t, partition p -> [p%16, 8*T + p//16]
        tmp = si.reshape(n_cores, 8, 16, K)          # p = s*16 + r
        blocks_idx.append(np.ascontiguousarray(
            tmp.transpose(0, 2, 3, 1)).reshape(n_cores, 16, 8 * K))
        blocks_mask.append(sm)
        off += K
    TOT = off
    idx16 = np.concatenate(blocks_idx, axis=2).astype(np.int16)
    idx16 = np.tile(idx16, (1, 8, 1))               # [n_cores, 128, 8*TOT]
    mask = np.concatenate(blocks_mask, axis=2).astype(np.float32)
    return pos, Ks, toff, TOT, calls, bases, idx16, mask


def _host_prep(x, edge_index, W1, att1_src, att1_dst, W2, att2_src, att2_dst):
    N, F = x.shape
    H, C = att1_src.shape
    HC = H * C
    NCLS = W2.shape[1]
    n_cores = N_CORES
    nch = -(-N // (n_cores * 128))
    npcp = nch * 128
    R = n_cores * npcp

    pos, Ks, toff, TOT, calls, bases, idx16, mask = _edge_plan(
        edge_index, N, n_cores, nch, npcp)

    # Folded attention-logit weight columns
    Wa_s = np.einsum("fhc,hc->fh", W1.reshape(F, H, C), att1_src).astype(np.float32)
    Wa_d = np.einsum("fhc,hc->fh", W1.reshape(F, H, C), att1_dst).astype(np.float32)
    W1e = np.ascontiguousarray(
        np.concatenate([W1, Wa_s, Wa_d], axis=1), dtype=np.float32)  # [F, 528]

    w2s = (W2 @ att2_src[0]).astype(np.float32)
    w2d = (W2 @ att2_dst[0]).astype(np.float32)
    W2e_flat = np.zeros((HC, 64), np.float32)
    W2e_flat[:, :NCLS] = W2
    W2e_flat[:, NCLS] = w2s
    W2e_flat[:, NCLS + 1] = w2d
    nslab = HC // 128
    W2e = np.ascontiguousarray(
        W2e_flat.reshape(nslab, 128, 64).transpose(1, 0, 2))  # [128, 4, 64]

    import ml_dtypes
    bf = ml_dtypes.bfloat16
    xtab = np.zeros((R, F), np.float32)
    xtab[pos[np.arange(N)]] = x
    xTp = np.ascontiguousarray(xtab.T).astype(bf)   # [F, R] permuted cols
    W1e = W1e.astype(bf)
    ident = np.eye(128, dtype=np.float32)

    cfg = dict(
        N=N, F=F, H=H, C=C, HC=HC, NCLS=NCLS, n_cores=n_cores,
        nch=nch, npcp=npcp, R=R, nslab=nslab,
        Ks=Ks, toff=toff, TOT=TOT, calls=calls, bases=bases, pos=pos,
        swdge_queues=4, p0_bufs=4, gt_bufs=3,
    )
    shared = dict(xTp=xTp, W1e=W1e, W2e=W2e, ident=ident)
    per_core = [
        dict(g1idx=idx16[k], mask=mask[k].astype(bf))
        for k in range(n_cores)
    ]
    return cfg, shared, per_core


# ----------------------------------------------------------------------------
# Device program
# ----------------------------------------------------------------------------

def _build_program(cfg):
    F, HC, NCLS = cfg["F"], cfg["HC"], cfg["NCLS"]
    n_cores, npcp, R = cfg["n_cores"], cfg["npcp"], cfg["R"]
    nslab, TOT = cfg["nslab"], cfg["TOT"]
    ROW1, ROW2 = 640, 128

    nc = bacc.Bacc("TRN2", target_bir_lowering=False, debug=False,
                   num_devices=n_cores,
                   num_swdge_queues=cfg.get("swdge_queues", 1))

    xTp = nc.dram_tensor("xTp", [F, R], BF16, kind="ExternalInput").ap()
    W1e = nc.dram_tensor("W1e", [F, HC + 16], BF16, kind="ExternalInput").ap()
    W2e = nc.dram_tensor("W2e", [128, nslab, 64], F32, kind="ExternalInput").ap()
    ident_d = nc.dram_tensor("ident", [128, 128], F32, kind="ExternalInput").ap()
    g1idx = nc.dram_tensor("g1idx", [128, 8 * TOT], I16,
                           kind="ExternalInput").ap()
    mask_d = nc.dram_tensor("mask", [128, TOT], BF16, kind="ExternalInput").ap()

    T1 = nc.dram_tensor("T1", [R, ROW1], BF16).ap()
    tb2_own = nc.dram_tensor("tb2_own", [npcp, ROW2], BF16).ap()
    tb2_full = nc.dram_tensor("tb2_full", [R, ROW2], BF16,
                              addr_space="Shared").ap()
    out2 = nc.dram_tensor("out2", [npcp, NCLS], F32, kind="ExternalOutput").ap()

    tensors = dict(xTp=xTp, W1e=W1e, W2e=W2e, ident=ident_d, g1idx=g1idx,
                   mask=mask_d, T1=T1, tb2_own=tb2_own, tb2_full=tb2_full,
                   out2=out2)
    repeat = cfg.get("repeat", 1)
    with tile.TileContext(nc) as tc:
        for _ in range(repeat):
            _emit(tc, cfg, tensors)
    nc.compile()
    return nc


def _emit(tc, cfg, t):
    nc = tc.nc
    H, HC, NCLS = cfg["H"], cfg["HC"], cfg["NCLS"]
    n_cores, nch, npcp, R = cfg["n_cores"], cfg["nch"], cfg["npcp"], cfg["R"]
    nslab = cfg["nslab"]
    NTB = R // 128

    with tc.tile_pool(name="consts", bufs=1) as cpool:
        W1e_sb = cpool.tile([128, HC + 16], BF16)
        nc.sync.dma_start(W1e_sb[:], t["W1e"][:, :])
        W2e_sb = cpool.tile([128, nslab, 64], F32)
        nc.sync.dma_start(W2e_sb[:], t["W2e"][:, :, :])
        identf_sb = cpool.tile([128, 128], F32)
        nc.sync.dma_start(identf_sb[:], t["ident"][:, :])
        ident_bf = cpool.tile([128, 128], BF16)
        nc.vector.tensor_copy(ident_bf[:], identf_sb[:])
        ald1_all = cpool.tile([128, NTB, H], F32)
        ald1_sb = cpool.tile([128, nch, H], F32)
        ald2_sb = cpool.tile([128, nch, 1], F32)

        # ---------------- Phase 0: permuted node table T1 ----------------
        with (
            tc.tile_pool(name="p0", bufs=cfg.get("p0_bufs", 3)) as pool,
            tc.tile_pool(name="p0ps", bufs=cfg.get("p0_bufs", 3),
                         space="PSUM") as pps,
        ):
            nblk = R // 512
            for i in range(nblk):
                xt = pool.tile([128, 512], BF16, tag="xt")
                nc.sync.dma_start(xt[:], t["xTp"][:, 512 * i: 512 * i + 512])
                rowB = pool.tile([128, 4, HC + 16], BF16, tag="rowB")
                for j in range(4):
                    ps = pps.tile([128, 1024], F32, tag="ps")
                    nc.tensor.matmul(ps[:, 0:HC], lhsT=xt[:, 128 * j: 128 * j + 128],
                                     rhs=W1e_sb[:, 0:HC], start=True, stop=True)
                    nc.tensor.matmul(ps[:, 512: 512 + 16],
                                     lhsT=xt[:, 128 * j: 128 * j + 128],
                                     rhs=W1e_sb[:, HC: HC + 16],
                                     start=True, stop=True)
                    nc.vector.tensor_copy(rowB[:, j, 0:HC], ps[:, 0:HC])
                    nc.scalar.copy(rowB[:, j, HC: HC + 16].bitcast(F32),
                                   ps[:, 512: 512 + H])
                    nc.scalar.copy(ald1_all[:, 4 * i + j, :],
                                   ps[:, 512 + H: 512 + 2 * H])
                # ACT HWDGE ring: don't serialize behind xt reads on SP ring
                nc.scalar.dma_start(
                    t["T1"][512 * i: 512 * i + 512, 0: HC + 16].rearrange(
                        "(j p) c -> p j c", p=128),
                    rowB[:],
                )

        pid = nc.partition_id()
        nc.sync.dma_start(ald1_sb[:], ald1_all[:, bass.ds(pid * nch, nch), :])

        if cfg.get("phases", "full") == "p0":
            return
        # ---------------- L1 edge phase ----------------
        _edge_phase(tc, cfg, layer=1, gather_src=t["T1"], grow=640,
                    idx_d=t["g1idx"], mask_d=t["mask"],
                    ald_sb=ald1_sb, identf_sb=identf_sb, ident_bf=ident_bf,
                    W2e_sb=W2e_sb, tb2_own=t["tb2_own"], out2=None,
                    ald2_cap=ald2_sb)

        if cfg.get("phases", "full") == "p0+l1":
            return
        # ---------------- allgather ----------------
        if cfg.get("no_collective"):
            nc.sync.dma_start(t["tb2_full"][0:npcp, :], t["tb2_own"][:, :])
        else:
            nc.gpsimd.collective_compute(
                "AllGather",
                OP.bypass,
                replica_groups=[list(range(n_cores))],
                ins=[t["tb2_own"][:, :]],
                outs=[t["tb2_full"][:, :]],
            )

        if cfg.get("phases", "full") == "p0+l1+ag":
            return
        # ---------------- L2 edge phase ----------------
        _edge_phase(tc, cfg, layer=2, gather_src=t["tb2_full"], grow=128,
                    idx_d=t["g1idx"], mask_d=t["mask"],
                    ald_sb=ald2_sb, identf_sb=identf_sb, ident_bf=ident_bf,
                    W2e_sb=None, tb2_own=None, out2=t["out2"])


def _edge_phase(tc, cfg, layer, gather_src, grow, idx_d, mask_d, ald_sb,
                identf_sb, ident_bf, W2e_sb, tb2_own, out2, ald2_cap=None):
    nc = tc.nc
    nch, H, HC, NCLS = cfg["nch"], cfg["H"], cfg["HC"], cfg["NCLS"]
    nslab = cfg["nslab"]
    Ks, toff, calls, bases = cfg["Ks"], cfg["toff"], cfg["calls"], cfg["bases"]
    HL = H if layer == 1 else 1      # heads this layer
    als_off = HC if layer == 1 else NCLS  # bf16 col of al_src f32 pairs

    TOT = cfg["TOT"]
    with (
        tc.tile_pool(name=f"gt{layer}", bufs=cfg.get("gt_bufs", 2)) as gpool,
        tc.tile_pool(name=f"meta{layer}", bufs=1) as mpool,
        tc.tile_pool(name=f"msg{layer}", bufs=4) as msgpool,
        tc.tile_pool(name=f"small{layer}", bufs=3) as smpool,
        tc.tile_pool(name=f"out{layer}", bufs=2) as opool,
        tc.tile_pool(name=f"ps_u{layer}", bufs=2, space="PSUM") as pp_u,
        tc.tile_pool(name=f"ps_tr{layer}", bufs=2, space="PSUM") as pp_tr,
        tc.tile_pool(name=f"ps_o{layer}", bufs=2, space="PSUM") as pp_o,
    ):
        # one bulk prefetch per layer keeps idx/mask DMAs off the per-chunk
        # gather dependency chain
        idx_all = mpool.tile([128, 8 * TOT], I16, tag="idxall")
        nc.sync.dma_start(idx_all[:], idx_d[:, :])
        msk_all = mpool.tile([128, TOT], BF16, tag="mskall")
        nc.sync.dma_start(msk_all[:], mask_d[:, :])
        for c in range(nch):
            K = Ks[c]
            off = toff[c]
            gt = gpool.tile([128, K, grow], BF16, tag="gt")
            nq = cfg.get("swdge_queues", 1)
            for (b0, b1, q) in calls[c]:
                nk = b1 - b0
                nc.gpsimd.dma_gather(
                    gt[:, b0:b1, :],
                    gather_src[bases[q]: bases[q] + SPAN, :],
                    idx_all[:, 8 * (off + b0): 8 * (off + b1)],
                    nk * 128, nk * 128, grow,
                    queue_num=_QCTR[0] % nq,
                )
                _QCTR[0] += 1
            if layer == 1 and cfg.get("l1_mode") == "gather":
                continue

            # p = exp(leakyrelu(al_src[src] + al_dst[dst])) * mask
            s_t = smpool.tile([128, K, HL], F32, tag="s")
            nc.vector.tensor_tensor(
                s_t[:],
                gt[:, :, als_off: als_off + 2 * HL].bitcast(F32),
                ald_sb[:, c, None, :].to_broadcast([128, K, HL]),
                op=OP.add,
            )
            l_t = smpool.tile([128, K, HL], F32, tag="l")
            nc.vector.scalar_tensor_tensor(
                l_t[:], s_t[:], 0.2, s_t[:], op0=OP.mult, op1=OP.max
            )
            p_t = smpool.tile([128, K, HL], F32, tag="p")
            nc.scalar.activation(p_t[:], l_t[:], AF.Exp)
            p_bf = smpool.tile([128, K, HL], BF16, tag="pbf")
            nc.vector.tensor_tensor(
                p_bf[:], p_t[:],
                msk_all[:, off: off + K, None].to_broadcast([128, K, HL]),
                op=OP.mult,
            )

            if layer == 1:
                ps_u = pp_u.tile([128, HC], F32, tag="u")
                for k in range(K):
                    msg = msgpool.tile([128, HC], BF16, tag="msg")
                    nc.vector.tensor_tensor(
                        msg[:].rearrange("p (h c) -> p h c", h=H),
                        gt[:, k, 0:HC].rearrange("p (h c) -> p h c", h=H),
                        p_bf[:, k, :, None].to_broadcast([128, H, HC // H]),
                        op=OP.mult,
                    )
                    nc.tensor.matmul(
                        ps_u[:], lhsT=ident_bf[:], rhs=msg[:],
                        start=(k == 0), stop=(k == K - 1),
                    )
                zr = smpool.tile([128, H], F32, tag="zr")
                nc.vector.tensor_reduce(
                    zr[:], p_bf[:].rearrange("p k h -> p h k"),
                    axis=AX.X, op=OP.add,
                )
                zb = smpool.tile([128, H], F32, tag="zb")
                nc.vector.tensor_scalar_max(zb[:], zr[:], 1e-30)
                rz = smpool.tile([128, H], F32, tag="rz")
                nc.vector.reciprocal(rz[:], zb[:])
                h2 = opool.tile([128, HC], F32, tag="h2")
                nc.vector.tensor_tensor(
                    h2[:].rearrange("p (h c) -> p h c", h=H),
                    ps_u[:].rearrange("p (h c) -> p h c", h=H),
                    rz[:, :, None].to_broadcast([128, H, HC // H]),
                    op=OP.mult,
                )
                h2r = opool.tile([128, HC], F32, tag="h2r")
                nc.scalar.activation(h2r[:], h2[:], AF.Relu)
                ps_o = pp_o.tile([128, 64], F32, tag="o")
                for j in range(nslab):
                    ps_tr = pp_tr.tile([128, 128], F32, tag="tr")
                    nc.tensor.transpose(
                        ps_tr[:], h2r[:, 128 * j: 128 * (j + 1)], identf_sb[:]
                    )
                    h2t = smpool.tile([128, 128], F32, tag="h2t")
                    nc.scalar.copy(h2t[:], ps_tr[:])
                    nc.tensor.matmul(
                        ps_o[:], lhsT=h2t[:], rhs=W2e_sb[:, j, :],
                        start=(j == 0), stop=(j == nslab - 1),
                    )
                trow = opool.tile([128, 128], BF16, tag="trow")
                nc.vector.tensor_copy(trow[:, 0:NCLS], ps_o[:, 0:NCLS])
                nc.scalar.copy(trow[:, NCLS: NCLS + 4].bitcast(F32),
                               ps_o[:, NCLS: NCLS + 2])
                nc.scalar.copy(ald2_cap[:, c, :], ps_o[:, NCLS + 1: NCLS + 2])
                nc.sync.dma_start(tb2_own[128 * c: 128 * (c + 1), :], trow[:])
            else:
                msg2 = msgpool.tile([128, K, NCLS], BF16, tag="msg2")
                nc.vector.tensor_tensor(
                    msg2[:], gt[:, :, 0:NCLS],
                    p_bf[:, :, 0, None].to_broadcast([128, K, NCLS]),
                    op=OP.mult,
                )
                u2 = smpool.tile([128, NCLS], F32, tag="u2")
                nc.vector.tensor_reduce(
                    u2[:], msg2[:].rearrange("p k f -> p f k"),
                    axis=AX.X, op=OP.add,
                )
                z2 = smpool.tile([128, 1], F32, tag="z2")
                nc.vector.tensor_reduce(
                    z2[:], p_bf[:, :, 0], axis=AX.X, op=OP.add,
                )
                zb2 = smpool.tile([128, 1], F32, tag="zb2")
                nc.vector.tensor_scalar_max(zb2[:], z2[:], 1e-30)
                rz2 = smpool.tile([128, 1], F32, tag="rz2")
                nc.vector.reciprocal(rz2[:], zb2[:])
                o2 = opool.tile([128, NCLS], F32, tag="o2")
                nc.vector.tensor_tensor(
                    o2[:], u2[:], rz2[:].to_broadcast([128, NCLS]), op=OP.mult,
                )
                nc.sync.dma_start(out2[128 * c: 128 * (c + 1), :], o2[:])


# ----------------------------------------------------------------------------
# PJRT execution (with on-device iteration chaining for timing)
# ----------------------------------------------------------------------------

def _pjrt_exec(nc, in_maps, n_cores, iters=1, reps=3):
    import jax
    import numpy as _np
    from jax.sharding import Mesh, PartitionSpec
    from jax.experimental.shard_map import shard_map
    from concourse import bass2jax as b2j
    from concourse import mybir as _mb

    b2j.install_neuronx_cc_hook()
    partition_name = (nc.partition_id_tensor.name
                      if nc.partition_id_tensor else None)
    in_names, out_names, out_avals, zero_outs = [], [], [], []
    for alloc in nc.m.functions[0].allocations:
        if not isinstance(alloc, _mb.MemoryLocationSet):
            continue
        name = alloc.memorylocations[0].name
        if alloc.kind == "ExternalInput":
            if name != partition_name:
                in_names.append(name)
        elif alloc.kind == "ExternalOutput":
            shape = tuple(alloc.tensor_shape)
            dtype = _mb.dt.np(alloc.dtype)
            out_names.append(name)
            out_avals.append(jax.core.ShapedArray(shape, dtype))
            zero_outs.append(_np.zeros(shape, dtype))
    n_params = len(in_names)
    all_in_names = in_names + out_names
    if partition_name is not None:
        all_in_names = all_in_names + [partition_name]

    def _body(*args):
        ins = list(args[:n_params])
        zo = list(args[n_params:])
        for _ in range(iters):
            operands = ins + zo
            if partition_name is not None:
                operands.append(b2j.partition_id_tensor())
            outs = _bass_exec_bind(b2j, operands, out_avals, all_in_names,
                                   out_names, nc)
            zo = list(outs)
        return tuple(zo)

    devices = jax.devices()[:n_cores]
    mesh = Mesh(_np.asarray(devices), ("core",))
    in_specs = (PartitionSpec("core"),) * (n_params + len(out_names))
    out_specs = (PartitionSpec("core"),) * len(out_names)
    sharded = jax.jit(shard_map(_body, mesh=mesh, in_specs=in_specs,
                                out_specs=out_specs, check_rep=False),
                      keep_unused=True)
    concat_in = [
        _np.concatenate([_np.asarray(in_maps[c][nm]) for c in range(n_cores)],
                        axis=0)
        for nm in in_names
    ]
    concat_zeros = [_np.zeros((n_cores * z.shape[0], *z.shape[1:]), z.dtype)
                    for z in zero_outs]
    import time as _time
    from jax.sharding import NamedSharding
    sh = NamedSharding(mesh, PartitionSpec("core"))
    dev_in = [jax.device_put(a, sh) for a in concat_in]
    dev_zeros = [jax.device_put(a, sh) for a in concat_zeros]
    jax.block_until_ready(dev_in + dev_zeros)
    out_arrs = sharded(*dev_in, *dev_zeros)
    jax.block_until_ready(out_arrs)
    times = []
    for _ in range(reps):
        t0 = _time.perf_counter()
        out_arrs = sharded(*dev_in, *dev_zeros)
        jax.block_until_ready(out_arrs)
        times.append(_time.perf_counter() - t0)
    dt = min(times)
    results = [
        {nm: _np.asarray(out_arrs[i]).reshape(n_cores, *out_avals[i].shape)[c]
         for i, nm in enumerate(out_names)}
        for c in range(n_cores)
    ]
    return results, dt


def _bass_exec_bind(b2j, operands, out_avals, in_names, out_names, nc):
    return b2j._bass_exec_p.bind(
        *operands,
        out_avals=tuple(out_avals),
        in_names=tuple(in_names),
        out_names=tuple(out_names),
        lowering_input_output_aliases=(),
        sim_require_finite=True,
        sim_require_nnan=True,
        nc=nc,
    )


# ----------------------------------------------------------------------------
# Entry point
# ----------------------------------------------------------------------------

_CACHE = {}


def _run(inputs, trace=False):
    x = np.asarray(inputs["x"], np.float32)
    edge_index = np.asarray(inputs["edge_index"], np.int32)
    W1 = np.asarray(inputs["W1"], np.float32)
    a1s = np.asarray(inputs["att1_src"], np.float32)
    a1d = np.asarray(inputs["att1_dst"], np.float32)
    W2 = np.asarray(inputs["W2"], np.float32)
    a2s = np.asarray(inputs["att2_src"], np.float32)
    a2d = np.asarray(inputs["att2_dst"], np.float32)
    b1 = np.asarray(inputs["b1"], np.float32)
    b2 = np.asarray(inputs["b2"], np.float32)
    assert not b1.any() and not b2.any(), "nonzero bias unsupported"

    key = hashlib.sha1(
        b"v2" + edge_index.tobytes() + np.int64(x.shape).tobytes()
    ).hexdigest()
    cfg, shared, per_core = _host_prep(x, edge_index, W1, a1s, a1d, W2, a2s, a2d)
    if key not in _CACHE:
        _CACHE[key] = _build_program(cfg)
    nc = _CACHE[key]

    in_maps = []
    for k in range(cfg["n_cores"]):
        m = dict(shared)
        m.update(per_core[k])
        in_maps.append(m)
    res = run_bass_kernel_spmd(nc, in_maps, list(range(cfg["n_cores"])),
                               trace=trace)
    out = gather_out([res.results[k]["out2"] for k in range(cfg["n_cores"])],
                     cfg)
    return out.astype(np.float32), res


def gather_out(outs, cfg):
    allrows = np.concatenate(outs, axis=0)          # [R, NCLS] permuted
    return allrows[cfg["pos"][: cfg["N"]]]


def kernel(**inputs):
    out, _ = _run(inputs, trace=False)
    return out
